# revision 56
# baseline (speedup 1.0000x reference)
"""Binarized 4-layer MLP (8192x784 -> 6144 -> 6144 -> 6144 -> 10, log_softmax)
on 8 Trainium2 NeuronCores, data-parallel over the batch.

Per-core dataflow (batch slice of 1024, feature-major activations [feat, batch]):
  fc1: x @ sign(w1).T as a 2-term fp16 hi/lo split of x stacked along the
       contraction dim (1568 rows = 12 full k-tiles + 32 leftover rows).
       fp16 upconverts losslessly to the PE's e10m11 internal format and the
       weights are exactly +-1, so this reproduces fp32 accuracy.
       Groups of 3 m-tiles run k-outer; the 32 leftover rows are handled by
       3 concurrent row-tiled matmuls (tile_position 0/32/64), costing one
       pass instead of three. xt streams in halves on two DMA queues so the
       PE starts on the first k-tile; fc1 weights stream per-k for the first
       two groups (gpsimd/scalar queues) and per-group afterwards.
  fc2/fc3: sign(h) @ sign(w).T in fp8e4 with DoubleRow perf mode. All products
       are +-1 and partial sums are small integers, so fp32 PSUM accumulation
       is bit-exact regardless of order.
  fc4: fused into the fc3 m-loop, single fp16 pass (w4 and h3 in fp16);
       b4 is accumulated into the logits via a rank-1 matmul.
  log_softmax: computed in the [10, batch] layout without transposes -
       class-sum via a ones-vector matmul, ln(sum) accumulated back into the
       logits PSUM via a rank-1 matmul with -1 weights, output DMAed straight
       from PSUM. Output is [10, batch]; the host transposes.
All PSUM rotates through one shared 6x[128,512] pool (+2 banks for the
logits), so there are no zone-recycle barriers at layer boundaries.
"""

import numpy as np
import ml_dtypes

import concourse.bass as bass
import concourse.mybir as mybir
from concourse import bacc
from concourse.tile import TileContext
from concourse.bass_utils import run_bass_kernel_spmd

dt = mybir.dt

CORES = 8
B = 8192
BC = B // CORES          # 1024 batch rows per core
DIN = 784
KT1 = 13                 # fc1 contraction tiles: 2*784 = 1568 padded to 1664
K1P = KT1 * 128
DH = 6144
MT = DH // 128           # 48 feature tiles
KB = DH // 256           # 24 DoubleRow contraction blocks
DOUT = 10
NH = BC // 512           # 2 moving halves of 512
G1 = 16                  # fc1 m-groups
GM = MT // G1            # 3 m-tiles per group

BF16 = ml_dtypes.bfloat16
FP8 = mybir.dt.np(dt.float8e4)

last_exec_time_ns = None


def _build_program():
    nc = bacc.Bacc("TRN2", target_bir_lowering=False, debug=False,
                   num_devices=CORES)

    xt = nc.dram_tensor("xt", [128, KT1, BC], dt.float16,
                        kind="ExternalInput").ap()
    w1t = nc.dram_tensor("w1t", [G1, 128, KT1, GM * 128], dt.float8e4,
                         kind="ExternalInput").ap()
    w2p = nc.dram_tensor("w2p", [MT, 128, KB, 2, 128], dt.float8e4,
                         kind="ExternalInput").ap()
    w3p = nc.dram_tensor("w3p", [MT, 128, KB, 2, 128], dt.float8e4,
                         kind="ExternalInput").ap()
    w4p = nc.dram_tensor("w4p", [128, MT, 32], dt.float16,
                         kind="ExternalInput").ap()
    b1p = nc.dram_tensor("b1p", [128, MT], dt.float32, kind="ExternalInput").ap()
    b2p = nc.dram_tensor("b2p", [128, MT], dt.float32, kind="ExternalInput").ap()
    b3p = nc.dram_tensor("b3p", [128, MT], dt.float32, kind="ExternalInput").ap()
    onesp = nc.dram_tensor("onesp", [DOUT, 512], dt.float16,
                           kind="ExternalInput").ap()
    negp = nc.dram_tensor("negp", [1, DOUT], dt.float16,
                          kind="ExternalInput").ap()
    selp = nc.dram_tensor("selp", [128, DOUT], dt.float16,
                          kind="ExternalInput").ap()
    b4c = nc.dram_tensor("b4c", [DOUT, 1], dt.float32,
                         kind="ExternalInput").ap()
    out = nc.dram_tensor("out", [DOUT, BC], dt.float32,
                         kind="ExternalOutput").ap()

    DR = mybir.MatmulPerfMode.DoubleRow
    AF = mybir.ActivationFunctionType

    with TileContext(nc) as tc:
        with tc.tile_pool(name="consts", bufs=1) as cpool, \
             tc.tile_pool(name="h1p", bufs=1) as h1pool, \
             tc.tile_pool(name="psp", bufs=6, space="PSUM") as pspool:
            # --- startup DMAs in fc1 consumption order: xt half-tiles mostly
            # on the fast sync queue (the scalar HWDGE queue is ~2.5x slower,
            # so it only carries a few early n=1 halves); fc1 g0/g1 weights
            # stream per-k in fp8 on gpsimd.
            # All fc1-only inputs live in fc1io, released before fc3. ---
            fc1io_stack = tc.tile_pool(name="fc1io", bufs=1)
            fpool = fc1io_stack.__enter__()
            # PE warm-up: junk matmuls on the ones tile (tiny DMA, lands at
            # the ~3.5us DMA-latency floor - earlier than any engine memset,
            # whose queue is blocked by the framework preamble until ~6us) so
            # the HAM clock gate reaches 8/8 before real data arrives
            ones_sb = cpool.tile([DOUT, 512], dt.float16)
            nc.sync.dma_start(out=ones_sb[:], in_=onesp[:])
            wps = pspool.tile([128, 512], dt.float32, tag="ps")
            NWARM = 2
            for i in range(NWARM):
                nc.tensor.matmul(wps[0:64, :], ones_sb[:, 0:64],
                                 ones_sb[:, 0:512],
                                 start=(i == 0), stop=(i == NWARM - 1))

            b1_sb = cpool.tile([128, MT], dt.float32)
            nc.scalar.dma_start(out=b1_sb[:], in_=b1p[:])
            xt_half = {}
            xt_tiles = {}
            scalar_n1 = {0, 2, 4, 6, 8}
            for k in range(KT1):
                tx = fpool.tile([128, BC], dt.float16, tag=f"xt_{k}")
                xt_tiles[k] = tx
                nc.sync.dma_start(out=tx[:, 0:512], in_=xt[:, k, 0:512])
                if k in scalar_n1:
                    nc.scalar.dma_start(out=tx[:, 512:1024],
                                        in_=xt[:, k, 512:1024])
                else:
                    nc.sync.dma_start(out=tx[:, 512:1024],
                                      in_=xt[:, k, 512:1024])
                for n in range(NH):
                    xt_half[(k, n)] = tx[:, n * 512:(n + 1) * 512]
            # fc1 group-0/1 weights, one DMA per k-tile in consumption order
            w1q0, w1q1 = {}, {}
            for k in range(KT1):
                tw = fpool.tile([128, GM * 128], dt.float8e4, tag=f"w1q0_{k}")
                nc.gpsimd.dma_start(out=tw[:], in_=w1t[0, :, k, :])
                w1q0[k] = tw
            for k in range(KT1):
                tw = fpool.tile([128, GM * 128], dt.float8e4, tag=f"w1q1_{k}")
                nc.gpsimd.dma_start(out=tw[:], in_=w1t[1, :, k, :])
                w1q1[k] = tw
            # small consts + first fc2 weight tile on sync (fits before fc2)
            w4_sb = cpool.tile([128, MT, 32], dt.float16)
            nc.sync.dma_start(out=w4_sb[:], in_=w4p[:])
            neg_sb = cpool.tile([1, DOUT], dt.float16)
            nc.sync.dma_start(out=neg_sb[:], in_=negp[:])
            sel_sb = cpool.tile([128, DOUT], dt.float16)
            nc.sync.dma_start(out=sel_sb[:], in_=selp[:])
            b4c_sb = cpool.tile([DOUT, 1], dt.float32)
            nc.sync.dma_start(out=b4c_sb[:], in_=b4c[:])
            w2f = cpool.tile([128, KB, 2, 128], dt.float8e4)
            nc.sync.dma_start(out=w2f[:], in_=w2p[0])
            warm = cpool.tile([1, 1], dt.float32)

            h1 = h1pool.tile([128, MT, BC], dt.float8e4)

            # ---------------- fc1 (k-outer groups of 3 m-tiles) --------------
            with tc.tile_pool(name="w1pool", bufs=3) as w1pool:
                for g in range(G1):
                    if g == 0:
                        def lhs1(k, mi, wk=w1q0):
                            return wk[k][:, mi * 128:(mi + 1) * 128]
                    elif g == 1:
                        def lhs1(k, mi, wk=w1q1):
                            return wk[k][:, mi * 128:(mi + 1) * 128]
                    else:
                        w1g = w1pool.tile([128, KT1, GM * 128], dt.float8e4,
                                          tag="w1")
                        nc.gpsimd.dma_start(out=w1g[:], in_=w1t[g])

                        def lhs1(k, mi, w1g=w1g):
                            return w1g[:, k, mi * 128:(mi + 1) * 128]
                    psums = [[pspool.tile([128, 512], dt.float32, tag="ps",
                                          name=f"ps_{mi}_{n}")
                              for n in range(NH)] for mi in range(GM)]
                    for k in range(KT1):
                        for mi in range(GM):
                            for n in range(NH):
                                nc.tensor.matmul(
                                    psums[mi][n][:, :],
                                    lhs1(k, mi),
                                    xt_half[(k, n)],
                                    start=(k == 0),
                                    stop=(k == KT1 - 1),
                                )
                    for mi in range(GM):
                        m = g * GM + mi
                        for n in range(NH):
                            nc.scalar.sign(h1[:, m, n * 512:(n + 1) * 512],
                                           psums[mi][n][:, :],
                                           bias=b1_sb[:, m:m + 1])

            # release the fc1 input tiles (frees SBUF for fc3's h3 pool)
            fc1io_stack.__exit__(None, None, None)

            # late prefetches: queue behind the fc1 weight stream on gpsimd
            # (needed only at fc2/fc3 start, long after the queue drains)
            b2_sb = cpool.tile([128, MT], dt.float32)
            nc.gpsimd.dma_start(out=b2_sb[:], in_=b2p[:])
            b3_sb = cpool.tile([128, MT], dt.float32)
            nc.gpsimd.dma_start(out=b3_sb[:], in_=b3p[:])
            w3f = cpool.tile([128, KB, 2, 128], dt.float8e4)
            nc.gpsimd.dma_start(out=w3f[:], in_=w3p[0])

            # ---------------- fc2 ----------------
            with tc.tile_pool(name="h2p", bufs=1) as h2pool:
                h2 = h2pool.tile([128, MT, BC], dt.float8e4)
                with tc.tile_pool(name="w2pool", bufs=3) as w2pool:
                    for m in range(MT):
                        if m == 0:
                            wsb = w2f
                        else:
                            wsb = w2pool.tile([128, KB, 2, 128], dt.float8e4,
                                              tag="w2")
                            nc.sync.dma_start(out=wsb[:], in_=w2p[m])
                        for n in range(NH):
                            psum = pspool.tile([128, 512], dt.float32,
                                               tag="ps")
                            for b in range(KB):
                                nc.tensor.matmul(
                                    psum[:, :],
                                    wsb[:, b],
                                    h1[:, 2 * b:2 * b + 2,
                                       n * 512:(n + 1) * 512],
                                    start=(b == 0),
                                    stop=(b == KB - 1),
                                    perf_mode=DR,
                                )
                            nc.scalar.sign(h2[:, m, n * 512:(n + 1) * 512],
                                           psum[:, :], bias=b2_sb[:, m:m + 1])

                # ---------------- fc3 + fused fc4 ----------------
                with tc.tile_pool(name="lgp", bufs=1, space="PSUM") as lgp, \
                     tc.tile_pool(name="smp", bufs=1) as smp:
                    # fc4 runs 4-way column-tiled: m-tile m accumulates into
                    # PE column group m%4 (partitions 32j..32j+9), so 4
                    # matmuls run concurrently; the 4 partial logits are
                    # reduced across partition groups in the tail
                    lg_psum = lgp.tile([128, BC], dt.float32)
                    with tc.tile_pool(name="w3pool", bufs=3) as w3pool, \
                         tc.tile_pool(name="h3pool", bufs=18) as h3pool:
                        h3_tiles = [None] * MT

                        def fc4_mms(m, n):
                            j = 32 * (m % 4)
                            nc.tensor.matmul(
                                lg_psum[j:j + 32, n * 512:(n + 1) * 512],
                                w4_sb[:, m, :],
                                h3_tiles[m][:, n * 512:(n + 1) * 512],
                                start=(m < 4),
                                stop=(m >= MT - 4),
                                tile_position=(0, j),
                            )

                        for m in range(MT):
                            if m == 0:
                                wsb = w3f
                            else:
                                wsb = w3pool.tile([128, KB, 2, 128],
                                                  dt.float8e4, tag="w3")
                                nc.sync.dma_start(out=wsb[:], in_=w3p[m])
                            t_h3 = h3pool.tile([128, BC], dt.float16, tag="h3")
                            for n in range(NH):
                                psum = pspool.tile([128, 512], dt.float32,
                                                   tag="ps")
                                for b in range(KB):
                                    nc.tensor.matmul(
                                        psum[:, :],
                                        wsb[:, b],
                                        h2[:, 2 * b:2 * b + 2,
                                           n * 512:(n + 1) * 512],
                                        start=(b == 0),
                                        stop=(b == KB - 1),
                                        perf_mode=DR,
                                    )
                                sl = slice(n * 512, (n + 1) * 512)
                                nc.scalar.activation(t_h3[:, sl], psum[:, :],
                                                     AF.Identity,
                                                     bias=b3_sb[:, m:m + 1])
                                nc.vector.tensor_scalar(t_h3[:, sl],
                                                        t_h3[:, sl], 1.0, -1.0,
                                                        mybir.AluOpType.min,
                                                        mybir.AluOpType.max)
                            h3_tiles[m] = t_h3
                            # fc4 batched every 8 m-tiles (fewer stationary /
                            # perf-mode switches on the PE), pipelined one m
                            # behind so the PE never waits on this m's ACT/DVE
                            if m % 8 == 7 and m >= 15:
                                for n in range(NH):
                                    for mm in range(m - 15, m - 7):
                                        fc4_mms(mm, n)
                        # pre-load the Exp table while the PE runs the last
                        # fc4 batch (a function switch always reloads the
                        # table, so this hides the 1.3us load off the tail)
                        nc.scalar.activation(warm[:], b1_sb[0:1, 0:1], AF.Exp)
                        # last fc4 batch n-half-major: half 0's partials close
                        # early so its reduce/exp overlap half 1's matmuls
                        red_sb = smp.tile([128, BC], dt.float16)
                        lgt_ps = []
                        exp_sb = smp.tile([DOUT, BC], dt.float16)
                        for n in range(NH):
                            sl = slice(n * 512, (n + 1) * 512)
                            for mm in range(MT - 8, MT):
                                fc4_mms(mm, n)
                            # reduce the 4 column-group partials to logits
                            # via a 0/1-selector matmul
                            nc.scalar.copy(red_sb[:, sl], lg_psum[:, sl])
                            lp = pspool.tile([128, 512], dt.float32, tag="ps",
                                             name=f"lgt{n}")
                            nc.tensor.matmul(lp[0:DOUT, :], sel_sb[:, :],
                                             red_sb[:, sl],
                                             start=True, stop=False)
                            lgt_ps.append(lp)
                            nc.scalar.activation(exp_sb[:, sl],
                                                 lp[0:DOUT, :], AF.Exp,
                                                 bias=b4c_sb[:, 0:1])

                    # ------------- log_softmax, [10, batch] layout -----------
                    # logits are tiny (|x| < ~4), so exp never overflows and
                    # the max-subtraction pass is unnecessary.
                    ln_row = smp.tile([1, BC], dt.float16)
                    sums = []
                    for n in range(NH):
                        sum_ps = pspool.tile([128, 512], dt.float32, tag="ps")
                        nc.tensor.matmul(sum_ps[0:1, :],
                                         ones_sb[:, 0:1],
                                         exp_sb[:, n * 512:(n + 1) * 512],
                                         start=True, stop=True)
                        sums.append(sum_ps)
                    for n in range(NH):
                        nc.scalar.activation(ln_row[:, n * 512:(n + 1) * 512],
                                             sums[n][0:1, :], AF.Ln)
                    # accumulate -ln(sum) into the reduced logits via a
                    # rank-1 matmul with -1 weights, add b4 in one DVE op,
                    # DMA out on the fast sync queue
                    res = smp.tile([DOUT, BC], dt.float32)
                    for n in range(NH):
                        sl = slice(n * 512, (n + 1) * 512)
                        nc.tensor.matmul(lgt_ps[n][0:DOUT, :],
                                         neg_sb[:, :],
                                         ln_row[:, sl],
                                         start=False, stop=True,
                                         skip_group_check=True)
                        nc.vector.tensor_scalar(res[:, sl],
                                                lgt_ps[n][0:DOUT, :],
                                                b4c_sb[:, 0:1], None,
                                                mybir.AluOpType.add)
                        nc.sync.dma_start(out=out[:, sl], in_=res[:, sl])

    nc.compile()
    return nc


def _pack_inputs(x, w1, b1, w2, b2, w3, b3, w4, b4):
    """Host-side packing into the device layouts. Shared tensors are packed
    once; only xt differs per core."""
    f32 = np.float32
    f16 = np.float16
    x = np.asarray(x, f32).reshape(B, DIN)

    # fc1 weights: sign(w1).T stacked twice (hi/lo terms share the weights),
    # padded to [1664, 6144], fp8 (+-1 is exact), layout [g, p, k, m]
    s1 = np.sign(np.asarray(w1, f32))                       # [DH, DIN]
    s1t = np.zeros((K1P, DH), f32)
    s1t[:DIN] = s1.T
    s1t[DIN:2 * DIN] = s1.T
    w1t = np.ascontiguousarray(
        s1t.reshape(KT1, 128, G1, GM * 128).transpose(2, 1, 0, 3)).astype(FP8)

    def pack_dr(w):
        # sign(w).T -> [mo, p, b, i, m'] DoubleRow stationary layout
        st = np.sign(np.asarray(w, f32)).T                  # [in, out]
        r = st.reshape(KB, 2, 128, MT, 128)                 # [b, i, p, mo, m']
        return np.ascontiguousarray(r.transpose(3, 2, 0, 1, 4)).astype(FP8)

    w2p = pack_dr(w2)
    w3p = pack_dr(w3)

    # fc4 weights: w4.T in fp16 padded to 32 cols (zeros init the unused
    # PSUM partitions of each column group), layout [p, j, c]
    w4t = np.asarray(w4, f32).T.astype(f16)                 # [DH, DOUT]
    w4pad = np.zeros((DH, 32), f16)
    w4pad[:, :DOUT] = w4t
    w4p = np.ascontiguousarray(w4pad.reshape(MT, 128, 32).transpose(1, 0, 2))

    def pack_b(b):
        return np.ascontiguousarray(np.asarray(b, f32).reshape(MT, 128).T)

    b1p, b2p, b3p = pack_b(b1), pack_b(b2), pack_b(b3)
    b4c = np.asarray(b4, f32).reshape(DOUT, 1)
    onesp = np.ones((DOUT, 512), f16)
    negp = np.full((1, DOUT), -1.0, f16)
    selp = np.zeros((128, DOUT), f16)
    for j in range(4):
        for c in range(DOUT):
            selp[32 * j + c, c] = 1.0

    shared = {"w1t": w1t, "w2p": w2p, "w3p": w3p, "w4p": w4p,
              "b1p": b1p, "b2p": b2p, "b3p": b3p, "b4c": b4c,
              "onesp": onesp, "negp": negp, "selp": selp}

    # per-core x: fp16 hi/lo split stacked along contraction, layout [p, k, n]
    in_maps = []
    for c in range(CORES):
        xc = x[c * BC:(c + 1) * BC]                         # [BC, DIN]
        hi = xc.astype(f16)
        lo = (xc - hi.astype(f32)).astype(f16)
        arr = np.zeros((K1P, BC), f16)
        arr[:DIN] = hi.T
        arr[DIN:2 * DIN] = lo.T
        xt = np.ascontiguousarray(arr.reshape(KT1, 128, BC).transpose(1, 0, 2))
        in_maps.append({"xt": xt, **shared})
    return in_maps


_cached_nc = None


def kernel(x, w1, b1, w2, b2, w3, b3, w4, b4):
    global _cached_nc, last_exec_time_ns
    import os
    trace = bool(int(os.environ.get("KERNEL_TRACE", "0")))
    if _cached_nc is None:
        _cached_nc = _build_program()
    in_maps = _pack_inputs(x, w1, b1, w2, b2, w3, b3, w4, b4)
    res = run_bass_kernel_spmd(_cached_nc, in_maps, list(range(CORES)),
                               trace=trace)
    last_exec_time_ns = res.exec_time_ns
    return np.ascontiguousarray(np.concatenate(
        [res.results[c]["out"].T for c in range(CORES)], axis=0))
